# revision 27
# baseline (speedup 1.0000x reference)
"""Multi-head attention (B=1, S=4096, D=1024, H=16) on 8 TRN2 NeuronCores.

Sharding: tensor-parallel over heads — 2 heads per core. W_q/W_k/W_v are
column-sharded (rows of the torch-style weight), W_o row-sharded; each core
produces a partial output [S, D] (fp16) and the host sums the 8 partials.

Per-core dataflow (fp16 data, fp32 PSUM accumulation). ScalarE is the pacing
engine: 256 exp instructions of [128,1024] at ~1us each (~255us floor); all
other engines are kept off that critical path:
  1. QcT/KcT = [128(2h*64), 4096]: Qc^T = Wq_c @ q^T via chunked matmuls.
  2. Vc (natural [s, j] layout) + augmented ones column per head (gives the
     softmax denominator for free through the AV matmul).
  3. Attention in S^T layout: St[sk, sq] = Kh^T.T @ Qh^T (two heads packed via
     PE row-tiling), exp via ScalarE directly from PSUM (scale=1/8 and a
     constant -8 shift folded in; max-subtraction skipped — scores bounded).
  4. AV in NATURAL layout: ctx[sq128, 65] += E_chunk^T.T @ [Vh | 1] with the
     E tile as the stationary operand (65-row moving) — ~2.5x cheaper on PE
     than the U-layout AV (stationary loads are hidden by streaming). The 8
     per-block accumulation chains share 2 PSUM banks, so only the FIRST
     chunk's k=0 matmul carries start=True (HW first_mm clears has_written
     for the whole bank); AV consumption lags the exp by AV_LAG iterations
     via a global pending queue that crosses block/repeat boundaries.
  5. Normalize with per-partition reciprocal multiply (denominator = col 64),
     PE-transpose the natural ctx into CT = ctx^T [128, S] fp16 (both heads
     stacked), out-proj = single 128-contraction matmul per 512-col half;
     partials leave as fp16 (halves the output DMA).
  6. Chores for block b (normalize/transpose/out-proj) run inside block
     b+1's loop; the next REPEAT's weights/K/V/Q loads and projections are
     prefetched through blocks 6-7 so every block 0 starts hot (this is what
     the repeat-differential benchmark measures as steady-state throughput).

Exp is split across TWO engines to get ScalarE off the critical path
(PE, at ~220us of matmuls, becomes the pacer): ~200 tiles stay on ScalarE
(native exp, ~1.04us each) and 6 tiles per block (DVE_SK) go to a 2-op DVE
chain (~2.4us each): (1) tensor_scalar ti = rne_u16(st*A+B) straight from
fp32 PSUM (uint16 output saturates negative scores to 0 -> z=+0, replacing
the old relu clamp), then (2) ONE fused custom-DVE op (FEXP_ANT, 8 ALU
stages) that extracts the octave distance arithmetically via a magic-add
(v + 1.5*2^33 rounds to multiples of 1024; ABSOLUTE_DIFF gives
a = min(m, 1024-m)) and applies the same quadratic mantissa correction as
EXP_CORR (a*(1024-a) == m*(1024-m) by symmetry): e = z - z*a*C*(1024-a).
Max rel err ~0.6% on normal-range E (HW-validated); subnormal-E tiles are
approximated/zeroed, which is harmless under softmax (ratio <= 2^-11 to
the row max). GPSIMD cannot read PSUM on TRN2 (BIR verifier), so the only
Pool assists are SBUF-side: vaug memsets and the CT copy chore.

Softmax bias subtlety: a nonzero b_q adds a per-COLUMN (sk) offset
c_h[sk] = K_h[sk]·b_q_h to the scores (row-constant terms cancel in softmax).
Handled exactly by scaling V rows and the ones-column by exp(c_h[sk])
(host passes the tiny exp(c) vectors; all-ones when b_q == 0, via the "bq"
build variant). b_v/b_o contribute a constant row vector w_o@b_v + b_o added
on the host (softmax rows sum to 1).
"""

import sys

sys.path.insert(0, "/opt/trn_rl_repo")

import numpy as np

import concourse.bass as bass
import concourse.mybir as mybir
import concourse.tile as tile
from concourse import bacc
from concourse import bass_utils
from concourse.masks import make_identity
from concourse import dve_ops as _dve_ops
from concourse.dve_spec import Spec as _Spec, Src0, Src1, C0, C1, C2, Bin as _Bin, lower as _lower
from concourse.dve_uop import AluOp as _AluOp, DveOpSpec as _DveOpSpec

D = 1024
S = 4096
H = 16
HD = 64
NCORES = 8
HPC = H // NCORES  # heads per core = 2
JW = HPC * HD  # per-core projected width = 128
F16 = mybir.dt.float16
F32 = mybir.dt.float32
EXP_SHIFT = 8.0  # exp(s/8 - 8): keeps E in fp16 range; cancels in softmax

NSQ = S // 512  # 8 query blocks of 512
NSK = S // 128  # 32 key blocks of 128
NDC = D // 128  # 8 contraction chunks

# ---- DVE fast-exp: uint16 Schraudolph + one fused correction op ----
# ts1: ti = rne_u16(st*A + B)  (uint16 saturation clamps st < -18 to z=+0)
# FEXP_ANT (one DVE instruction, 8 ALU stages, fp32 pipeline):
#   t2 = v + 1.5*2^33 (rne to multiples of 1024); ef = t2 - 1.5*2^33
#   a  = |v - ef| = min(m, 1024-m);  e = z - (a*C)*z*(1024 - a)
# == z * (1 + C*m*(m-1024)) (the EXP_CORR quadratic, by symmetry of the
# parabola), ~0.6% max rel err vs exp(st/8 - 8) on normal-range E.
_LOG2E = float(np.log2(np.e))
EXP_A = 128.0 * _LOG2E
EXP_B = 1024.0 * (15 - 8 * _LOG2E) - 4.0
I16 = mybir.dt.int16
U16 = mybir.dt.uint16
FEXP_S0 = 2.0962e-7  # quadratic correction C (in mantissa units)
FEXP_S1 = 1.5 * 2.0**33  # magic: rne to multiples of 1024
FEXP_IMM2 = 1024.0
# DVE-offloaded exp tiles; constraint sk <= 31 + AV_LAG - DVE_LAG so the
# prev block's DVE AVs land before the sk==AV_LAG normalize chores.
DVE_SK = (11, 15, 19, 23, 26, 29)
DVE_LAG = 10
AV_LAG = 8
assert max(DVE_SK) <= 31 + AV_LAG - DVE_LAG and min(DVE_SK) > AV_LAG
OP0 = max(10, AV_LAG + 5 + ((AV_LAG + 5) % 2))  # first outproj sk (even), past transposes
assert OP0 + 15 <= 31


def _fexp_ref(in0, in1, s0, s1, imm2):
    f32 = np.float32
    v = in0.astype(f32)
    z = in1.astype(f32)
    t2 = (v + f32(s1)).astype(f32)
    ef = (t2 - f32(s1)).astype(f32)
    a = np.abs((v - ef).astype(f32))
    q2 = ((a * f32(s0)).astype(f32) * z).astype(f32)
    q4 = (q2 * (f32(imm2) - a).astype(f32)).astype(f32)
    return (z - q4).astype(np.float32)


def _register_fexp():
    if "FEXP_ANT" in _dve_ops._SUB_OPCODE_FOR_NAME:
        return next(o for o in _dve_ops.OPS if o.name == "FEXP_ANT")
    t2 = Src0 + C1
    ef = t2 - C1
    a = _Bin(_AluOp.ABSOLUTE_DIFF, Src0, ef)
    q2 = (a * C0) * Src1
    q4 = q2 * _Bin(_AluOp.SUBTRACT, C2, a)
    spec = _Spec(body=Src1 - q4, reference=_fexp_ref)
    shas = {}
    for ver in ("v3", "v4"):
        tmp = _DveOpSpec(name="FEXP_ANT", uops=_lower(spec, ver=ver), rd1_en=True)
        shas[ver] = tmp.sha(ver)
    op = _dve_ops.DveOp("FEXP_ANT", spec, subdim=False, uops_sha=shas)
    _dve_ops.OPS.append(op)
    _dve_ops.CUSTOM_DVE_SPECS[op.name] = op.spec
    _dve_ops._SUB_OPCODE_FOR_NAME[op.name] = (
        max(_dve_ops._SUB_OPCODE_FOR_NAME.values()) + 1
    )
    return op


FEXP_ANT = _register_fexp()


def _emit(tc: tile.TileContext, repeats: int = 1, bench_mode: int = 0, variant: str = "full"):
    nc = tc.nc
    # bench_mode 2: inputs live in Internal DRAM (garbage values) so the
    # benchmark call carries no host->device payload.
    ik = "Internal" if bench_mode == 2 else "ExternalInput"
    qT = nc.dram_tensor("qT", (D, S), F16, kind=ik).ap()
    kT = nc.dram_tensor("kT", (D, S), F16, kind=ik).ap()
    vT = nc.dram_tensor("vT", (D, S), F16, kind=ik).ap()
    wqT = nc.dram_tensor("wqT", (D, JW), F16, kind=ik).ap()
    wkT = nc.dram_tensor("wkT", (D, JW), F16, kind=ik).ap()
    wvT = nc.dram_tensor("wvT", (D, JW), F16, kind=ik).ap()
    woT = nc.dram_tensor("woT", (JW, D), F16, kind=ik).ap()
    # expc[p, 2*i+h] = exp(c_h[i*128+p]) for s-chunk i, head h (ones if b_q=0)
    expc = nc.dram_tensor("expc", (128, 2 * NSK), F32, kind=ik).ap()
    if bench_mode:
        outp = nc.dram_tensor("outp", (S, D), F16, kind="Internal").ap()
        dummy = nc.dram_tensor("bench_out", (1, 128), F32, kind="ExternalOutput").ap()
    else:
        outp = nc.dram_tensor("outp", (S, D), F16, kind="ExternalOutput").ap()
        dummy = None

    Exp = mybir.ActivationFunctionType.Exp
    with (
        tc.tile_pool(name="consts", bufs=1) as cpool,
        tc.tile_pool(name="weights", bufs=2) as wpool,
        tc.tile_pool(name="big", bufs=2) as big,
        tc.tile_pool(name="chunks", bufs=3) as chunks,
        tc.tile_pool(name="qchunks", bufs=3) as qchunks,
        tc.tile_pool(name="stp", bufs=2, space="PSUM") as stp,
        tc.tile_pool(name="ctxp", bufs=1, space="PSUM") as ctxp,
        tc.tile_pool(name="mixp", bufs=2, space="PSUM") as mixp,
        tc.tile_pool(name="ep", bufs=14) as ep,
        tc.tile_pool(name="smallp", bufs=2) as smallp,
        tc.tile_pool(name="ctxnp", bufs=2) as ctxnp,
        tc.tile_pool(name="ostagep", bufs=6) as ostagep,
        tc.tile_pool(name="tip", bufs=3) as tip,
    ):
        pools = (wpool, big, chunks, qchunks, stp, ctxp, mixp, ep, smallp, ctxnp, ostagep, tip)
        # ---- compile-time constants (shared by all repeats) ----
        ident = cpool.tile([128, 128], F16)
        make_identity(nc, ident)
        negshift_sb = cpool.tile([128, 1], F32)
        nc.gpsimd.memset(negshift_sb[:], -EXP_SHIFT)
        # tiny dummy exp: pulls the ~2.7us ACT_TABLE_LOAD off the critical
        # path of the first real exp (runs during the DMA/projection head)
        warm_sb = cpool.tile([128, 1], F16)
        nc.scalar.activation(warm_sb[:], negshift_sb[:], Exp, scale=0.125)
        consts = (ident, negshift_sb)

        # cross-repeat pipeline state: chores for block b run inside block
        # b+1's loop, including across the repeat boundary; AV matmuls are
        # consumed lag iterations after their exp by global iteration index.
        state = {"prev": None, "pending": [], "bcount": {}, "g": 0}

        for rep in range(repeats):
            _emit_once(
                tc, qT, kT, vT, wqT, wkT, wvT, woT, expc, outp,
                pools, consts, state, variant, rep, repeats,
            )

        # ---- final drain + epilogue for the very last block ----
        _drain_avs(nc, state, None)
        prev = state["prev"]
        if prev is not None:
            _chore_recip(nc, prev)
            for j in range(8):
                _chore_tsmul(nc, prev, j)
            for c in range(4):
                tp = _chore_transpose(nc, mixp, ident, prev, c)
                _chore_ctcopy(nc, prev, c, tp)
            for scl in range(4):
                for nh in range(2):
                    _chore_outproj(nc, mixp, ostagep, outp, prev, scl, nh)

        if dummy is not None:
            with tc.tile_pool(name="dummyp", bufs=1) as dp:
                dt_ = dp.tile([1, 128], F32)
                nc.gpsimd.memset(dt_[:], 1.0)
                nc.sync.dma_start(dummy[:, :], dt_[:])


def _emit_av(nc, k, e_t, ctxPs, vaug, last):
    # One start=True per PSUM bank (first chunk, k=0): on HW first_mm clears
    # has_written for the whole bank, so a per-chunk start would erase
    # sibling chunks' k=0 sums. Chunks 1-3 first-write with start=False
    # (overwrite-or-accumulate-onto-cleared - correct either way).
    for h in range(2):
        for c in range(4):
            nc.tensor.matmul(
                ctxPs[h][:, c * 66 : c * 66 + 65],
                e_t[:, h * 512 + c * 128 : h * 512 + (c + 1) * 128],
                vaug[:, k * 130 + h * 65 : k * 130 + (h + 1) * 65],
                start=(k == 0 and c == 0),
                stop=(last and c == 3),
                skip_group_check=True,
            )


def _drain_avs(nc, state, cur_g):
    rest = []
    for ent in state["pending"]:
        ready, k, e_t, ctxPs, vaug, bkey = ent
        if cur_g is None or ready <= cur_g:
            cnt = state["bcount"]
            cnt[bkey] = cnt.get(bkey, 0) + 1
            _emit_av(nc, k, e_t, ctxPs, vaug, last=(cnt[bkey] == NSK))
        else:
            rest.append(ent)
    state["pending"][:] = rest


def _chore_recip(nc, prev):
    ctxPs, r = prev["ctxPs"], prev["r"]
    for h in range(2):
        for c in range(4):
            j = h * 4 + c
            nc.vector.reciprocal(r[:, j : j + 1], ctxPs[h][:, c * 66 + 64 : c * 66 + 65])


def _chore_tsmul(nc, prev, j):
    ctxPs, r, ctxn = prev["ctxPs"], prev["r"], prev["ctxn"]
    h, c = divmod(j, 4)
    # ctxn is chunk-major: [sq128, c*128 + h*64] so one [128,128] transpose
    # per sq-chunk covers both heads (full partitions)
    nc.vector.tensor_scalar_mul(
        ctxn[:, c * 128 + h * 64 : c * 128 + h * 64 + 64],
        ctxPs[h][:, c * 66 : c * 66 + 64],
        r[:, j : j + 1],
    )


def _chore_transpose(nc, mixp, ident, prev, c):
    tp = mixp.tile([128, 128], F16, tag="mix", name="tp")
    nc.tensor.transpose(tp[:], prev["ctxn"][:, c * 128 : (c + 1) * 128], ident[:])
    return tp


def _chore_ctcopy(nc, prev, c, tp):
    sq = prev["sq"]
    nc.vector.tensor_copy(
        prev["CT"][:, sq * 512 + c * 128 : sq * 512 + (c + 1) * 128], tp[:]
    )


def _chore_outproj(nc, mixp, ostagep, outp, prev, scl, nh):
    s0 = prev["sq"] * 4 + scl
    scs = slice(s0 * 128, (s0 + 1) * 128)
    po = mixp.tile([128, 512], F32, tag="mix", name="po")
    nc.tensor.matmul(
        po[:],
        prev["CT"][:, scs],
        prev["wo_sb"][:, nh * 512 : (nh + 1) * 512],
        start=True,
        stop=True,
    )
    ost = ostagep.tile([128, 512], F16, tag="ost", name="ost")
    nc.vector.tensor_copy(ost[:], po[:])
    nc.sync.dma_start(outp[scs, nh * 512 : (nh + 1) * 512], ost[:])


def _emit_once(tc, qT, kT, vT, wqT, wkT, wvT, woT, expc, outp, pools, consts, state, variant, rep, repeats):
    """Software-pipelined emission of one full attention pass. ScalarE (exp,
    ~1us per (sq,sk) tile) is the pacing engine. The normalize/transpose/
    out-proj of block b runs inside block b+1's loop (crossing repeat
    boundaries), and the NEXT repeat's weights/K/V/Q loads + projections are
    prefetched during this repeat's blocks 6-7, so its block 0 starts hot."""
    nc = tc.nc
    Exp = mybir.ActivationFunctionType.Exp
    (wpool, big, chunks, qchunks, stp, ctxp, mixp, ep, smallp, ctxnp, ostagep, tip) = pools
    ident, negshift_sb = consts
    use_expc = variant == "bq"

    def alloc_res():
        res = {}
        res["wq_sb"] = wpool.tile([128, NDC * JW], F16, tag="wq", name="wq_sb")
        res["wk_sb"] = wpool.tile([128, NDC * JW], F16, tag="wk", name="wk_sb")
        res["wv_sb"] = wpool.tile([128, NDC * JW], F16, tag="wv", name="wv_sb")
        res["wo_sb"] = wpool.tile([128, D], F16, tag="wo", name="wo_sb")
        res["QcT"] = big.tile([128, S], F16, tag="qct", name="QcT")
        res["KcT"] = big.tile([128, S], F16, tag="kct", name="KcT")
        res["vaug"] = big.tile([128, NSK * 130], F16, tag="vaug", name="vaug")
        if use_expc:
            res["expc_sb"] = wpool.tile([128, 2 * NSK], F32, tag="expc", name="expc_sb")
            res["expc16"] = wpool.tile([128, 2 * NSK], F16, tag="expc16", name="expc16")
        return res

    def dma_weights(res):
        # one 3D-AP DMA per weight: [1024, JW] DRAM -> [128, NDC*JW] SBUF
        # (row c*128+p -> partition p, free c*JW+j). 24 DMAs -> 3: the SP
        # sequencer's ~650ns per-DMA issue cost was serializing the prefetch.
        for nm, src in (("wq_sb", wqT), ("wk_sb", wkT), ("wv_sb", wvT)):
            nc.sync.dma_start(
                res[nm][:].rearrange("p (c j) -> p c j", c=NDC),
                src.rearrange("(c p) j -> p c j", c=NDC),
            )
        nc.sync.dma_start(res["wo_sb"][:], woT[:, :])
        if use_expc:
            nc.sync.dma_start(res["expc_sb"][:], expc[:, :])
            nc.vector.tensor_copy(res["expc16"][:], res["expc_sb"][:])

    def vaug_init(res):
        # b_q == 0: expc scaling is identity; ones-columns come from one big
        # memset and vproj writes plain copies
        if not use_expc:
            nc.gpsimd.memset(res["vaug"][:], 1.0)

    def dma_kvblock(b):
        # single batched DMA per tensor: [1024, 512] DRAM slice -> one
        # [128, NDC*512] SBUF tile (chunk-major free dim)
        cs = slice(b * 512, (b + 1) * 512)
        kt_t = chunks.tile([128, NDC * 512], F16, tag="kchunk", name="kt_t")
        nc.sync.dma_start(
            kt_t[:].rearrange("p (c j) -> p c j", c=NDC),
            kT.rearrange("(c p) s -> p c s", c=NDC)[:, :, cs],
        )
        vt_t = chunks.tile([128, NDC * 512], F16, tag="vchunk", name="vt_t")
        nc.sync.dma_start(
            vt_t[:].rearrange("p (c j) -> p c j", c=NDC),
            vT.rearrange("(c p) s -> p c s", c=NDC)[:, :, cs],
        )
        kts = [kt_t[:, c * 512 : (c + 1) * 512] for c in range(NDC)]
        vts = [vt_t[:, c * 512 : (c + 1) * 512] for c in range(NDC)]
        return kts, vts

    def kproj_mms(res, b, kts):
        cs = slice(b * 512, (b + 1) * 512)
        kp = mixp.tile([128, 512], F32, tag="mix", name="kp")
        for c in range(NDC):
            nc.tensor.matmul(
                kp[:],
                res["wk_sb"][:, c * JW : (c + 1) * JW],
                kts[c][:],
                start=(c == 0),
                stop=(c == NDC - 1),
            )
        nc.vector.tensor_copy(res["KcT"][:, cs], kp[:])

    def vproj_mms(res, b, vts, half=None):
        vaug = res["vaug"]
        rng = range(4) if half is None else range(half * 2, half * 2 + 2)
        for ii in rng:
            i = b * 4 + ii
            vpt = mixp.tile([128, 512], F32, tag="mix", name="vpt")
            vps = vpt[:, 0:JW]
            for c in range(NDC):
                nc.tensor.matmul(
                    vps,
                    vts[c][:, ii * 128 : (ii + 1) * 128],
                    res["wv_sb"][:, c * JW : (c + 1) * JW],
                    start=(c == 0),
                    stop=(c == NDC - 1),
                )
            base = i * 130
            if use_expc:
                expc_sb, expc16 = res["expc_sb"], res["expc16"]
                nc.vector.tensor_scalar_mul(
                    vaug[:, base : base + 64], vps[:, 0:64], expc_sb[:, 2 * i : 2 * i + 1]
                )
                nc.vector.tensor_copy(
                    vaug[:, base + 64 : base + 65], expc16[:, 2 * i : 2 * i + 1]
                )
                nc.vector.tensor_scalar_mul(
                    vaug[:, base + 65 : base + 129],
                    vps[:, 64:128],
                    expc_sb[:, 2 * i + 1 : 2 * i + 2],
                )
                nc.vector.tensor_copy(
                    vaug[:, base + 129 : base + 130], expc16[:, 2 * i + 1 : 2 * i + 2]
                )
            else:
                nc.vector.tensor_copy(vaug[:, base : base + 64], vps[:, 0:64])
                nc.vector.tensor_copy(vaug[:, base + 65 : base + 129], vps[:, 64:128])

    def dma_qblock(sq):
        cs = slice(sq * 512, (sq + 1) * 512)
        qt_t = qchunks.tile([128, NDC * 512], F16, tag="qchunk", name="qt_t")
        nc.sync.dma_start(
            qt_t[:].rearrange("p (c j) -> p c j", c=NDC),
            qT.rearrange("(c p) s -> p c s", c=NDC)[:, :, cs],
        )
        return [qt_t[:, c * 512 : (c + 1) * 512] for c in range(NDC)]

    def qproj_mms(res, sq, qtiles):
        qp = mixp.tile([128, 512], F32, tag="mix", name="qp")
        for c in range(NDC):
            nc.tensor.matmul(
                qp[:],
                res["wq_sb"][:, c * JW : (c + 1) * JW],
                qtiles[c][:],
                start=(c == 0),
                stop=(c == NDC - 1),
            )
        nc.vector.tensor_copy(res["QcT"][:, sq * 512 : (sq + 1) * 512], qp[:])

    # ---- this repeat's resources: prefetched by the previous repeat, or
    # built cold (first repeat) through block 0's iterations ----
    if state.get("next_res") is not None:
        res = state.pop("next_res")
        head_done = True
    else:
        res = alloc_res()
        dma_weights(res)
        vaug_init(res)
        qtiles0 = dma_qblock(0)
        kts0, vts0 = dma_kvblock(0)
        kproj_mms(res, 0, kts0)
        qproj_mms(res, 0, qtiles0)
        vproj_mms(res, 0, vts0)
        head_done = False
    CT = big.tile([128, S], F16, tag="ct", name="CT")
    QcT, KcT, vaug = res["QcT"], res["KcT"], res["vaug"]
    prefetch = rep + 1 < repeats
    nres = None
    nkv = {}

    for sq in range(NSQ):
        sqs = slice(sq * 512, (sq + 1) * 512)
        ctxPa = ctxp.tile([128, 264], F32, tag="ca", name="ctxPa")
        ctxPb = ctxp.tile([128, 264], F32, tag="cb", name="ctxPb")
        ctxPs = (ctxPa, ctxPb)
        bkey = len(state["bcount"])

        qtiles = None
        kvts = {}
        for sk in range(NSK):
            sks = slice(sk * 128, (sk + 1) * 128)
            st = stp.tile([128, 1024], F32, name="st")
            nc.tensor.matmul(
                st[:, 0:512],
                KcT[0:64, sks],
                QcT[0:64, sqs],
                start=True,
                stop=True,
                tile_position=(0, 0),
            )
            nc.tensor.matmul(
                st[:, 512:1024],
                KcT[64:128, sks],
                QcT[64:128, sqs],
                start=True,
                stop=True,
                tile_position=(64, 0),
            )
            use_dve = (sk in DVE_SK) and variant != "scalar"
            e_t = ep.tile([128, 1024], F16, tag="e", name="e_t")
            if use_dve:
                ti = tip.tile([128, 1024], F16, tag="ti", name="ti")
                nc.vector.tensor_scalar(
                    ti.bitcast(U16), st[:], EXP_A, EXP_B,
                    mybir.AluOpType.mult, mybir.AluOpType.add,
                )
                nc.vector._custom_dve(
                    FEXP_ANT, out=e_t[:], in0=ti.bitcast(U16), in1=ti[:],
                    s0=FEXP_S0, s1=FEXP_S1, imm2=FEXP_IMM2,
                )
            else:
                nc.scalar.activation(e_t[:], st[:], Exp, scale=0.125, bias=negshift_sb[:])

            # ---- interleaved chores ----
            prev = state["prev"]
            if sq == 0 and not head_done and sk < 28:
                # cold start (first repeat): stream K/V loads + projections
                # through block 0's iterations
                b = sk // 4 + 1
                if sk % 4 == 0:
                    kvts[b] = dma_kvblock(b)
                elif sk % 4 == 1:
                    kproj_mms(res, b, kvts[b][0])
                elif sk % 4 == 2:
                    vproj_mms(res, b, kvts[b][1], half=0)
                elif sk % 4 == 3:
                    vproj_mms(res, b, kvts.pop(b)[1], half=1)
            if prev is not None:
                # prev block's final AVs drain during sk 0..AV_LAG-1 here
                # (SCALAR tiles: sk+AV_LAG-32 <= AV_LAG-1 always; DVE tiles
                # need sk <= 31+AV_LAG-DVE_LAG — enforced on DVE_SK). The
                # normalize chores run at EXACTLY sk==AV_LAG: after all prev
                # AVs have landed, but emitted before this block's k=0 AV
                # (start=True, bank-clears the shared ctxP banks) drains at
                # this same iteration's end.
                if sk == AV_LAG:
                    _chore_recip(nc, prev)
                    for j in range(8):
                        _chore_tsmul(nc, prev, j)
                elif AV_LAG + 1 <= sk <= AV_LAG + 4:
                    # transpose+copy emitted as an adjacent pair: the mix slot
                    # is read back immediately, so the 2-buf rotation stays
                    # safe no matter what else allocates around it
                    c0 = sk - (AV_LAG + 1)
                    tp = _chore_transpose(nc, mixp, ident, prev, c0)
                    _chore_ctcopy(nc, prev, c0, tp)
                elif OP0 <= sk < OP0 + 16 and (sk - OP0) % 2 == 0:
                    idx = (sk - OP0) // 2
                    _chore_outproj(nc, mixp, ostagep, outp, prev, idx // 2, idx % 2)
            if prefetch and sq >= 6:
                # build the NEXT repeat's resources during blocks 6-7
                if sq == 6 and sk == 0:
                    nres = alloc_res()
                    dma_weights(nres)
                elif sq == 6 and sk == 1:
                    vaug_init(nres)
                bofs = 0 if sq == 6 else 4
                b = None
                if sk in (2, 6, 10, 14):
                    b = bofs + (sk - 2) // 4
                    nkv[b] = dma_kvblock(b)
                elif sk in (4, 8, 12, 16):
                    b = bofs + (sk - 4) // 4
                    kproj_mms(nres, b, nkv[b][0])
                elif sk in (5, 9, 13, 17):
                    b = bofs + (sk - 5) // 4
                    vproj_mms(nres, b, nkv[b][1], half=0)
                elif sk in (7, 11, 15, 18):
                    b = bofs + ((sk - 7) // 4 if sk != 18 else 3)
                    vproj_mms(nres, b, nkv.pop(b)[1], half=1)
                elif sq == 7 and sk == 20:
                    nkv["q"] = dma_qblock(0)
                elif sq == 7 and sk == 24:
                    qproj_mms(nres, 0, nkv.pop("q"))
            if sk == 1 and sq + 1 < NSQ:
                qtiles = dma_qblock(sq + 1)
            if sk == 7 and sq + 1 < NSQ:
                qproj_mms(res, sq + 1, qtiles)

            lag = DVE_LAG if use_dve else AV_LAG
            g = state["g"]
            state["pending"].append((g + lag, sk, e_t, ctxPs, vaug, bkey))
            _drain_avs(nc, state, g)
            state["g"] = g + 1

        r = smallp.tile([128, 8], F32, tag="r", name="r")
        ctxn = ctxnp.tile([128, 512], F16, tag="ctxn", name="ctxn")
        state["prev"] = {
            "sq": sq, "ctxPs": ctxPs, "r": r, "ctxn": ctxn, "CT": CT,
            "wo_sb": res["wo_sb"],
        }

    if prefetch:
        state["next_res"] = nres

    if variant == "debug":
        nc_ = tc.nc
        prev = state["prev"]
        _drain_avs(nc, state, None)
        ctxPd = nc_.dram_tensor("ctxPd", (128, 528), F32, kind="ExternalOutput").ap()
        with tc.tile_pool(name="dbgp", bufs=1) as dbgp:
            dbg_sb = dbgp.tile([128, 528], F32)
            nc_.vector.tensor_copy(dbg_sb[:, 0:264], prev["ctxPs"][0][:])
            nc_.vector.tensor_copy(dbg_sb[:, 264:528], prev["ctxPs"][1][:])
            nc_.sync.dma_start(ctxPd[:, :], dbg_sb[:])
        QcTd = nc_.dram_tensor("QcTd", (128, S), F16, kind="ExternalOutput").ap()
        KcTd = nc_.dram_tensor("KcTd", (128, S), F16, kind="ExternalOutput").ap()
        vaugd = nc_.dram_tensor("vaugd", (128, NSK * 130), F16, kind="ExternalOutput").ap()
        nc_.sync.dma_start(QcTd[:, :], QcT[:])
        nc_.sync.dma_start(KcTd[:, :], KcT[:])
        nc_.sync.dma_start(vaugd[:, :], vaug[:])


_CACHE = {}


def _build(repeats: int = 1, bench_mode: int = 0, variant: str = "full"):
    key = (repeats, bench_mode, variant)
    if key in _CACHE:
        return _CACHE[key]
    nc = bacc.Bacc("TRN2", target_bir_lowering=False, debug=False, num_devices=NCORES)
    with tile.TileContext(nc) as tc:
        _emit(tc, repeats=repeats, bench_mode=bench_mode, variant=variant)
    nc.compile()
    _CACHE[key] = nc
    return nc


def _prep_inputs(q, k, v, w_q, b_q, w_k, b_k, w_v, b_v, w_o, b_o):
    """Build the 8 per-core input maps (and the host-side output correction)."""
    q2 = np.asarray(q, np.float32).reshape(S, D)
    k2 = np.asarray(k, np.float32).reshape(S, D)
    v2 = np.asarray(v, np.float32).reshape(S, D)
    qTh = np.ascontiguousarray(q2.T).astype(np.float16)
    kTh = np.ascontiguousarray(k2.T).astype(np.float16)
    vTh = np.ascontiguousarray(v2.T).astype(np.float16)

    in_maps = []
    for c in range(NCORES):
        rows = slice(c * JW, (c + 1) * JW)
        m = {
            "qT": qTh,
            "kT": kTh,
            "vT": vTh,
            "wqT": np.ascontiguousarray(np.asarray(w_q)[rows, :].T).astype(np.float16),
            "wkT": np.ascontiguousarray(np.asarray(w_k)[rows, :].T).astype(np.float16),
            "wvT": np.ascontiguousarray(np.asarray(w_v)[rows, :].T).astype(np.float16),
            "woT": np.ascontiguousarray(np.asarray(w_o)[:, rows].T).astype(np.float16),
        }
        # per-column softmax offset from b_q (exact): c_h[j] = K_h[j] . b_q_h
        ex = np.ones((128, 2 * NSK), np.float32)
        if np.any(np.asarray(b_q) != 0.0):
            for h in range(HPC):
                hrows = slice(c * JW + h * HD, c * JW + (h + 1) * HD)
                u = np.asarray(w_k)[hrows, :].T @ np.asarray(b_q)[hrows]  # [D]
                ch = k2 @ u + float(np.asarray(b_k)[hrows] @ np.asarray(b_q)[hrows])
                # scores are scaled by 1/sqrt(HD) before exp, so the offset is too
                ch = ch / np.sqrt(HD)
                ex[:, h::2] = (
                    np.exp(ch.astype(np.float64)).astype(np.float32).reshape(NSK, 128).T
                )
        m["expc"] = ex
        in_maps.append(m)

    corr = (np.asarray(w_o, np.float64) @ np.asarray(b_v, np.float64)) + np.asarray(
        b_o, np.float64
    )
    return in_maps, corr.astype(np.float32)


def kernel_with_results(trace=False, **inputs):
    import os
    variant = "bq" if np.any(np.asarray(inputs["b_q"]) != 0.0) else os.environ.get("KVARIANT", "full")
    nc = _build(variant=variant)
    in_maps, corr = _prep_inputs(**inputs)
    res = bass_utils.run_bass_kernel_spmd(
        nc, in_maps, core_ids=list(range(NCORES)), trace=trace
    )
    out = np.zeros((S, D), np.float32)
    for c in range(NCORES):
        out += res.results[c]["outp"].astype(np.float32)
    out += corr[None, :]
    return out.reshape(1, S, D), res


def kernel(**inputs):
    out, _ = kernel_with_results(trace=False, **inputs)
    return out

